# revision 6
# baseline (speedup 1.0000x reference)
"""Bass/Trainium2 kernel for nn_Apply2DTform: batched 2D affine warp with
round-based bilinear sampling, data-parallel over 8 NeuronCores (4 images each).

Per output pixel (i, j) of image b the reference samples the image at
  x = 255.5*(M00*ax[i] + M01*ay[j] + V0 + 1)   (height coordinate)
  y = 255.5*(M10*ax[i] + M11*ay[j] + V1 + 1)   (width coordinate)
with x0 = round(x), x1 = x0+1 (clipped to the zero-padded 513x513 image) and
bilinear-style weights that can be negative.  We reproduce this exactly with a
2-row clamped gather (rows rc, rc+1 with rc = clip(x0, 0, 510)) and edge-aware
weights that fold the zero-padding into the weights:
  x0 in [0,510]: (a0, a1) = (x0+1-x, x-x0)          rows (x0, x0+1)
  x0 == 511:     (a0, a1) = (0, 512-x)              rows (510, 511)
  else:          (0, 0)
(same for y -> (b0, b1), cols (qc, qc+1)); out = sum_uv a_u*b_v*I[rc+u, qc+v].

On-device per tile of 128 output rows x 512 cols: DVE computes x, y, the
weights and int32 entry offsets into a host-prewoven pair-image (entry
(r, q) = [Img[r, q], Img[r+1, q]], so 12 contiguous floats = the full 2x2
patch).  The hardware's indirect DMA supports one offset per partition per
instruction, so a dynamic For_i loop issues one 128-row gather per output
column; DVE then does the weighted combine into an NHWC tile that is DMAed
out.  Corner order within an entry pair: +0 I00, +3 I10, +6 I01, +9 I11.
"""
import os
import sys
import numpy as np
from contextlib import ExitStack

if "/opt/trn_rl_repo" not in sys.path:
    sys.path.insert(0, "/opt/trn_rl_repo")

import concourse.bass as bass
import concourse.tile as tile
from concourse import bacc, mybir
from concourse.bass import IndirectOffsetOnAxis
from concourse.bass_utils import run_bass_kernel_spmd

f32 = mybir.dt.float32
i32 = mybir.dt.int32
Alu = mybir.AluOpType

N_CORES = 8
B_FULL = 32
B_CORE = B_FULL // N_CORES  # 4 images per core
H = W = 512
HW = H * W                  # 262144 pixels per image
P = 128                     # partitions / output rows per tile
NT = H // P                 # 4 row-tiles per image
RNE = 12582912.0            # 1.5 * 2**23: (x + RNE) - RNE == round-half-even(x)

# Exact float32 values of jnp.linspace(-1.0, 1.0, 512) (== reference ax/ay).
_AX_HEX = (
    "000080bf80ff7ebf00ff7dbf80fe7cbf00fe7bbf7efd7abffefc79bf7efc78bffefb77bf7efb76bffefa75bf7efa74bffcf973bf7cf972bffcf871bf7cf870bffcf76fbf7af76ebffaf66dbf7af66cbffaf56bbf7af56abffaf469bf78f468bff8f367bf78f366bff8f265bf78f264bff8f163bf76f162bff6f061bf76f060bf"
    "f6ef5fbf76ef5ebff6ee5dbf74ee5cbff4ed5bbf74ed5abff4ec59bf74ec58bff4eb57bf72eb56bff2ea55bf72ea54bff2e953bf72e952bff2e851bf70e850bff0e74fbf70e74ebff0e64dbf70e64cbff0e54bbf6ee54abfeee449bf6ee448bfeee347bf6ee346bfeee245bf6ce244bfece143bf6ce142bfece041bf6ce040bf"
    "ecdf3fbf6adf3ebfeade3dbf6ade3cbfeadd3bbf6add3abfeadc39bf68dc38bfe8db37bf68db36bfe8da35bf68da34bfe8d933bf66d932bfe6d831bf66d830bfe6d72fbf66d72ebfe6d62dbf64d62cbfe4d52bbf64d52abfe4d429bf64d428bfe4d327bf62d326bfe2d225bf62d224bfe2d123bf62d122bfe2d021bf60d020bf"
    "e0cf1fbf60cf1ebfe0ce1dbf60ce1cbfe0cd1bbf5ecd1abfdecc19bf5ecc18bfdecb17bf5ecb16bfdeca15bf5cca14bfdcc913bf5cc912bfdcc811bf5cc810bfdcc70fbf5ac70ebfdac60dbf5ac60cbfdac50bbf5ac50abfdac409bf58c408bfd8c307bf58c306bfd8c205bf58c204bfd8c103bf56c102bfd6c001bf56c000bf"
    "acbfffbeac7ffdbe54bffbbe547ff9befcbef7befc7ef5bea4bef3bea47ef1be4cbeefbe4c7eedbef4bdebbef47de9be9cbde7be9c7de5be44bde3be447de1beecbcdfbeec7cddbe94bcdbbe947cd9be3cbcd7be3c7cd5bee4bbd3bee47bd1be8cbbcfbe8c7bcdbe34bbcbbe347bc9bedcbac7bedc7ac5be84bac3be847ac1be"
    "2cbabfbe2c7abdbed4b9bbbed479b9be7cb9b7be7c79b5be24b9b3be2479b1beccb8afbecc78adbe74b8abbe7478a9be1cb8a7be1c78a5bec4b7a3bec477a1be6cb79fbe6c779dbe14b79bbe147799bebcb697bebc7695be64b693be647691be0cb68fbe0c768dbeb4b58bbeb47589be5cb587be5c7585be04b583be047581be"
    "58e97ebe58e97abea8e876bea8e872bef8e76ebef8e76abe48e766be48e762be98e65ebe98e65abee8e556bee8e552be38e54ebe38e54abe88e446be88e442bed8e33ebed8e33abe28e336be28e332be78e22ebe78e22abec8e126bec8e122be18e11ebe18e11abe68e016be68e012beb8df0ebeb8df0abe08df06be08df02be"
    "b0bcfdbdb0bbf5bdb0baedbdb0b9e5bdb0b8ddbdb0b7d5bdb0b6cdbdb0b5c5bdb0b4bdbdb0b3b5bdb0b2adbdb0b1a5bdb0b09dbdb0af95bdb0ae8dbdb0ad85bd60597bbd60576bbd60555bbd60534bbd60513bbd604f2bbd604d1bbd604b0bbdc092f6bcc08ed6bcc08ab6bcc08696bc8005ecbb80fdabbb00ec57bb00acd7ba"
    "00acd73a00ec573b80fdab3b8005ec3bc086963cc08ab63cc08ed63cc092f63c604b0b3d604d1b3d604f2b3d60513b3d60534b3d60555b3d60576b3d60597b3db0ad853db0ae8d3db0af953db0b09d3db0b1a53db0b2ad3db0b3b53db0b4bd3db0b5c53db0b6cd3db0b7d53db0b8dd3db0b9e53db0baed3db0bbf53db0bcfd3d"
    "08df023e08df063eb8df0a3eb8df0e3e68e0123e68e0163e18e11a3e18e11e3ec8e1223ec8e1263e78e22a3e78e22e3e28e3323e28e3363ed8e33a3ed8e33e3e88e4423e88e4463e38e54a3e38e54e3ee8e5523ee8e5563e98e65a3e98e65e3e48e7623e48e7663ef8e76a3ef8e76e3ea8e8723ea8e8763e58e97a3e58e97e3e"
    "0475813e04b5833e5c75853e5cb5873eb475893eb4b58b3e0c768d3e0cb68f3e6476913e64b6933ebc76953ebcb6973e1477993e14b79b3e6c779d3e6cb79f3ec477a13ec4b7a33e1c78a53e1cb8a73e7478a93e74b8ab3ecc78ad3eccb8af3e2479b13e24b9b33e7c79b53e7cb9b73ed479b93ed4b9bb3e2c7abd3e2cbabf3e"
    "847ac13e84bac33edc7ac53edcbac73e347bc93e34bbcb3e8c7bcd3e8cbbcf3ee47bd13ee4bbd33e3c7cd53e3cbcd73e947cd93e94bcdb3eec7cdd3eecbcdf3e447de13e44bde33e9c7de53e9cbde73ef47de93e"
    "f4bdeb3e4c7eed3e4cbeef3ea47ef13ea4bef33efc7ef53efcbef73e547ff93e54bffb3eac7ffd3eacbfff3e56c0003ed6c0013e56c1023ed6c1033e58c2043ed8c2053e58c3063ed8c3073e58c4083ed8c4093e5ac50a3edac50b3e5ac60c3edac60d3e5ac70e3edcc70f3e5cc8103edcc8113e5cc9123edcc9133e5cca143e"
    "deca153e5ecb163edecb173e5ecc183ededc193e5ecd1a3ee0cd1b3e60ce1c3ee0ce1d3e60cf1e3ee0cf1f3e60d0203ee2d0213e62d1223ee2d1233e62d2243ee2d2253e62d3263ee4d3273e64d4283ee4d4293e64d52a3ee4d52b3e64d62c3ee6d62d3e66d72e3ee6d72f3e66d8303ee6d8313e66d9323ee8d9333e68da343e"
    "e8da353e68db363ee8db373e68dc383eeadc393e6add3a3eeadd3b3e6ade3c3eeade3d3e6adf3e3eecdf3f3e6ce0403eece0413e6ce1423eece1433e6ce2443eeee2453e6ee3463eeee3473e6ee4483eeee4493e6ee54a3ef0e54b3e70e64c3ef0e64d3e70e74e3ef0e74f3e70e8503ef2e8513e72e9523ef2e9533e72ea543e"
    "f2ea553e72eb563ef4eb573e74ec583ef4ec593e74ed5a3ef4ed5b3e74ee5c3ef6ee5d3e76ef5e3ef6ef5f3e76f0603ef6f0613e76f1623ef8f1633e78f2643ef8f2653e78f3663ef8f3673e78f4683efaf4693e7af56a3efaf56b3e7af66c3efaf66d3e7af76e3efcf76f3e7cf8703efcf8713e7cf9723efcf9733e7afa743e"
    "fafa753e7afb763efefb773e7efc783efefc793e7efd7a3e00fe7b3e80fe7c3e00ff7d3e80ff7e3e0000803e"
)
AX_TABLE = np.frombuffer(bytes.fromhex(_AX_HEX), dtype="<f4").copy()
assert AX_TABLE.shape == (512,), AX_TABLE.shape


def _col(tf, b, k):
    """[128,1] AP for Tform scalar k of image b from the broadcast tile."""
    return tf[:, 6 * b + k : 6 * b + k + 1]


def build_program() -> bacc.Bacc:
    nc = bacc.Bacc("TRN2", target_bir_lowering=False, debug=False,
                   num_devices=N_CORES)

    pimg = nc.dram_tensor("PairImg", [B_CORE * HW, 6], f32,
                          kind="ExternalInput").ap()
    tfb = nc.dram_tensor("TformB", [P, 6 * B_CORE], f32, kind="ExternalInput").ap()
    axc = nc.dram_tensor("AxCol", [P, NT], f32, kind="ExternalInput").ap()
    ayr = nc.dram_tensor("AyRow", [P, W], f32, kind="ExternalInput").ap()
    out = nc.dram_tensor("Out", [B_CORE, H, W, 3], f32, kind="ExternalOutput").ap()


    with tile.TileContext(nc) as tc, ExitStack() as ctx:
        const_pool = ctx.enter_context(tc.tile_pool(name="const", bufs=1))
        img_pool = ctx.enter_context(tc.tile_pool(name="perimg", bufs=2))
        wpool = ctx.enter_context(tc.tile_pool(name="work", bufs=2))
        tpool = ctx.enter_context(tc.tile_pool(name="tmp", bufs=1))
        gpool = ctx.enter_context(tc.tile_pool(name="gath", bufs=2))
        opool = ctx.enter_context(tc.tile_pool(name="outp", bufs=2))

        tf = const_pool.tile([P, 6 * B_CORE], f32)
        nc.sync.dma_start(tf[:], tfb)
        axt = const_pool.tile([P, NT], f32)
        nc.sync.dma_start(axt[:], axc)
        ayt = const_pool.tile([P, W], f32)
        nc.sync.dma_start(ayt[:], ayr)

        for b in range(B_CORE):
            # per-image j-dependent parts: M01*ay[j], M11*ay[j]  [128, 512]
            bx = img_pool.tile([P, W], f32, tag="bx")
            nc.vector.tensor_scalar_mul(bx[:], ayt[:], _col(tf, b, 1))
            by = img_pool.tile([P, W], f32, tag="by")
            nc.vector.tensor_scalar_mul(by[:], ayt[:], _col(tf, b, 3))

            for t in range(NT):
                # per-tile i-dependent parts: M00*ax[i], M10*ax[i]  [128, 1]
                axi = tpool.tile([P, 1], f32, tag="axi")
                nc.vector.tensor_scalar_mul(axi[:], axt[:, t : t + 1], _col(tf, b, 0))
                ayi = tpool.tile([P, 1], f32, tag="ayi")
                nc.vector.tensor_scalar_mul(ayi[:], axt[:, t : t + 1], _col(tf, b, 2))

                def axis_weights(bj, ai, vcol, tag):
                    """x/y coordinate -> (clamped base index rc, weights a0, a1)."""
                    xs = tpool.tile([P, W], f32, tag=tag + "xs")
                    # (M01*ay[j] + M00*ax[i]) + V  -- matches ref association
                    nc.vector.tensor_scalar(xs[:], bj[:], ai[:], vcol,
                                            Alu.add, Alu.add)
                    x = tpool.tile([P, W], f32, tag=tag + "x")
                    nc.vector.tensor_scalar(x[:], xs[:], 1.0, 0.5, Alu.add, Alu.mult)
                    nc.vector.tensor_scalar(x[:], x[:], 511.0, None, Alu.mult)
                    r0 = tpool.tile([P, W], f32, tag=tag + "r0")
                    nc.vector.tensor_scalar(r0[:], x[:], RNE, RNE,
                                            Alu.add, Alu.subtract)
                    rc = tpool.tile([P, W], f32, tag=tag + "rc")
                    nc.vector.tensor_scalar(rc[:], r0[:], 0.0, 510.0,
                                            Alu.max, Alu.min)
                    # m_std = 1[0 <= r0 <= 510]
                    msa = tpool.tile([P, W], f32, tag=tag + "msa")
                    nc.vector.tensor_scalar(msa[:], r0[:], 0.0, None, Alu.is_ge)
                    ms = tpool.tile([P, W], f32, tag=tag + "ms")
                    nc.vector.tensor_scalar(ms[:], r0[:], 510.0, None, Alu.is_le)
                    nc.vector.tensor_tensor(ms[:], ms[:], msa[:], Alu.mult)
                    me = tpool.tile([P, W], f32, tag=tag + "me")
                    nc.vector.tensor_scalar(me[:], r0[:], 511.0, None, Alu.is_equal)
                    tt = tpool.tile([P, W], f32, tag=tag + "t")
                    nc.vector.tensor_tensor(tt[:], x[:], rc[:], Alu.subtract)
                    p1 = tpool.tile([P, W], f32, tag=tag + "p1")
                    nc.vector.tensor_tensor(p1[:], tt[:], ms[:], Alu.mult)
                    a0 = tpool.tile([P, W], f32, tag=tag + "a0")
                    nc.vector.tensor_tensor(a0[:], ms[:], p1[:], Alu.subtract)
                    # a1 = t*m_std + (2 - t)*m_edge
                    u = tpool.tile([P, W], f32, tag=tag + "u")
                    nc.vector.tensor_scalar(u[:], tt[:], -1.0, 2.0,
                                            Alu.mult, Alu.add)
                    nc.vector.tensor_tensor(u[:], u[:], me[:], Alu.mult)
                    a1 = tpool.tile([P, W], f32, tag=tag + "a1")
                    nc.vector.tensor_tensor(a1[:], p1[:], u[:], Alu.add)
                    return rc, a0, a1

                rc, a0, a1 = axis_weights(bx, axi, _col(tf, b, 4), "r")
                qc, b0, b1 = axis_weights(by, ayi, _col(tf, b, 5), "q")

                # pixel offsets: off0 = rc*512 + qc (+ b*H*W via element_offset)
                offf = tpool.tile([P, W], f32, tag="offf")
                nc.vector.tensor_scalar(offf[:], rc[:], 512.0, None, Alu.mult)
                nc.vector.tensor_tensor(offf[:], offf[:], qc[:], Alu.add)
                off0 = wpool.tile([P, W], i32, tag="off0")
                nc.vector.tensor_copy(off0[:], offf[:])

                # corner weights
                w00 = wpool.tile([P, W], f32, tag="w00")
                nc.vector.tensor_tensor(w00[:], a0[:], b0[:], Alu.mult)
                w01 = wpool.tile([P, W], f32, tag="w01")
                nc.vector.tensor_tensor(w01[:], a0[:], b1[:], Alu.mult)
                w10 = wpool.tile([P, W], f32, tag="w10")
                nc.vector.tensor_tensor(w10[:], a1[:], b0[:], Alu.mult)
                w11 = wpool.tile([P, W], f32, tag="w11")
                nc.vector.tensor_tensor(w11[:], a1[:], b1[:], Alu.mult)

                # gather: the HW indirect DMA takes one offset per partition
                # per instruction, so loop over output columns in blocks of
                # GB: stage 16 offsets (dynamic DVE read), issue 16 static
                # [128]-offset gathers of 12-float 2x2 patches, then store the
                # block into the dense gather buffer (dynamic DVE write).
                g0 = gpool.tile([P, W * 12], f32, tag="g0")
                GB = 16
                with tc.For_i(0, W, GB) as j0:
                    offs_blk = gpool.tile([P, GB], i32, tag="offsblk")
                    nc.vector.tensor_copy(offs_blk[:],
                                          off0[:, bass.ds(j0, GB)])
                    g_blk = gpool.tile([P, GB * 12], f32, tag="gblk")
                    for i in range(GB):
                        nc.gpsimd.indirect_dma_start(
                            g_blk[:, i * 12:(i + 1) * 12], None,
                            pimg,
                            IndirectOffsetOnAxis(ap=offs_blk[:, i:i + 1],
                                                 axis=0),
                            element_offset=b * HW * 6,
                        )
                    nc.vector.tensor_copy(g0[:, bass.ds(j0 * 12, GB * 12)],
                                          g_blk[:])

                # combine into NHWC output tile
                ot = opool.tile([P, W * 3], f32, tag="ot")
                ov = ot[:].rearrange("p (j c) -> p c j", c=3)
                g0v = g0[:].rearrange("p (j k) -> p k j", k=12)
                for c in range(3):
                    m1 = tpool.tile([P, W], f32, tag="m1")
                    nc.vector.tensor_tensor(m1[:], w00[:], g0v[:, c, :], Alu.mult)
                    m2 = tpool.tile([P, W], f32, tag="m2")
                    nc.vector.tensor_tensor(m2[:], w01[:], g0v[:, c + 6, :], Alu.mult)
                    nc.vector.tensor_tensor(m1[:], m1[:], m2[:], Alu.add)
                    m3 = tpool.tile([P, W], f32, tag="m3")
                    nc.vector.tensor_tensor(m3[:], w10[:], g0v[:, c + 3, :], Alu.mult)
                    m4 = tpool.tile([P, W], f32, tag="m4")
                    nc.vector.tensor_tensor(m4[:], w11[:], g0v[:, c + 9, :], Alu.mult)
                    nc.vector.tensor_tensor(m3[:], m3[:], m4[:], Alu.add)
                    nc.vector.tensor_tensor(ov[:, c, :], m1[:], m3[:], Alu.add)

                nc.sync.dma_start(
                    out[b].rearrange("h w c -> h (w c)")[t * P : (t + 1) * P, :],
                    ot[:],
                )

    nc.compile()
    return nc


_PROGRAM = None


def _get_program():
    global _PROGRAM
    if _PROGRAM is None:
        _PROGRAM = build_program()
    return _PROGRAM


def make_in_maps(Img: np.ndarray, Tform: np.ndarray):
    Img = np.ascontiguousarray(np.asarray(Img, dtype=np.float32))
    Tform = np.ascontiguousarray(np.asarray(Tform, dtype=np.float32))
    ax_col = np.ascontiguousarray(AX_TABLE.reshape(NT, P).T)       # [128, 4]
    ay_row = np.ascontiguousarray(np.tile(AX_TABLE[None, :], (P, 1)))  # [128, 512]
    # pair-image: entry (b, r, q) = [Img[b, r, q, :], Img[b, r+1, q, :]]
    # (row r+1 = zeros for r = 511; rc <= 510 so those entries are never read)
    PI = np.zeros((B_FULL, H, W, 6), np.float32)
    PI[:, :, :, 0:3] = Img
    PI[:, :H - 1, :, 3:6] = Img[:, 1:, :, :]
    in_maps = []
    for c in range(N_CORES):
        sh = slice(c * B_CORE, (c + 1) * B_CORE)
        tf_b = np.tile(Tform[sh].reshape(1, 6 * B_CORE), (P, 1))
        in_maps.append({
            "PairImg": np.ascontiguousarray(PI[sh].reshape(B_CORE * HW, 6)),
            "TformB": np.ascontiguousarray(tf_b),
            "AxCol": ax_col,
            "AyRow": ay_row,
        })
    return in_maps


def kernel(Img: np.ndarray, Tform: np.ndarray, _trace: bool = False):
    nc = _get_program()
    res = run_bass_kernel_spmd(nc, make_in_maps(Img, Tform),
                               core_ids=list(range(N_CORES)), trace=_trace)
    out = np.concatenate([res.results[c]["Out"] for c in range(N_CORES)], axis=0)
    if _trace:
        kernel.last_exec_time_ns = res.exec_time_ns
        kernel.last_results = res
    return out
